# revision 40
# baseline (speedup 1.0000x reference)
"""Self-contained Trainium2 Bass kernel for the sparse point-attention module.

Strategy: shard the point dimension n across the 8 NeuronCores (512 points
each, both batch entries on every core).  Each core gets the full `pos`
(tiny) so the KNN is purely local; everything else is data-parallel and no
collectives are needed.

Per-core pipeline, software-pipelined one KNN tile ahead (tile = 128
queries):
  - dneg[i,j] = 2 p_i.p_j - |p_j|^2 via one bf16 matmul with a hi/lo
    split of pos and |p|^2 (K=11, ~2^-18 accurate); top-16 neighbours via
    DVE max8 / max_index / match_replace (tie behaviour matches
    jax.lax.top_k).
  - neighbour POSITIONS are fetched with ONE ap_gather per tile running
    on ALL 8 gpsimd Q7 cores in parallel: the 16-row position table is
    replicated into all 8 partition-groups, the index matrix is block
    transposed on the PE (16x16 blocks, u16 biased into normal-bf16
    range), and each core gathers the 256 (query,slot) columns of its own
    16-query group.  ap_gather costs ~22ns per index PER CORE plus ~10us
    dispatch, so the 8-way split + one-call-per-tile is ~30x cheaper than
    the naive per-chunk 16-channel gather which serialized the whole
    kernel (its dispatch+loop latency was the actual bottleneck of the
    baseline, pacing it at ~22us per 512 columns).
  - per 512-column chunk: q/p1 matmuls use per-group stationary weights
    (zero rows except the group's 4 position rows); rr = (Wq@pos + bq -
    bk + 1) - Wk@key accumulates in PSUM; vpe = Wp2@pe1 + Wv@val
    accumulates in PSUM; a-branch MLP in bf16; softmax over the 16
    neighbours as (sum e*w)/(sum e).

Algebraic folds done on the host: BN (eval mode) into Wa1/Wp1+biases; the
pe-path first layer composed with the q conv (Wp1q = Wp1f@Wq acts on raw
positions); (bq - bk + 1) as a q bias row; bp2+bv ride through the softmax
into the output bias We@(bv+bp2) + be; ba2 dropped (softmax-invariant).
"""

import numpy as np
import ml_dtypes

BF16 = ml_dtypes.bfloat16

# ---- problem dimensions (hardcoded, must match the grader's inputs) ----
B = 2
CIN = 128
N = 4096
KK = 16          # neighbours
DIM = 256
PHID = 64
AHID = 1024
NCORES = 8
NLOC = N // NCORES
BN_EPS = 1e-5
NEG_BIG = -1e30


def _dims_full():
    return dict(B=B, CIN=CIN, N=N, KK=KK, DIM=DIM, PHID=PHID, AHID=AHID,
                NLOC=NLOC)


def build_nc(dims):
    """Build the (single, SPMD) Bass program for one core's shard."""
    import concourse.bass as bass
    import concourse.mybir as mybir
    import concourse.tile as tile
    from concourse import bacc
    from concourse.bass import ts

    fp32 = mybir.dt.float32
    bf16 = mybir.dt.bfloat16
    u16 = mybir.dt.uint16
    i16 = mybir.dt.int16
    AF = mybir.ActivationFunctionType
    OP = mybir.AluOpType
    AX = mybir.AxisListType

    Bn = dims["B"]; CINn = dims["CIN"]; Nn = dims["N"]; KKn = dims["KK"]
    DIMn = dims["DIM"]; PHIDn = dims["PHID"]; AHIDn = dims["AHID"]
    NLOCn = dims["NLOC"]

    QT = min(128, NLOCn)              # queries per KNN tile
    NQT = NLOCn // QT                 # KNN tiles per batch
    CHUNK = 512                       # matmul column chunk (n,k cols)
    CQ = CHUNK // KKn                 # queries per chunk (32)
    NCH_TILE = (QT * KKn) // CHUNK    # chunks per KNN tile
    NCH_D = Nn // 512                 # 512-col chunks of the distance row
    DM = DIMn // 128                  # feature tiles (2)
    AM = AHIDn // 128                 # a-hidden tiles (8)
    KA1 = DIMn // 128                 # contraction tiles for a1 (2)
    GP = QT // 16                     # gather groups (Q7 cores) per tile
    GW = 16 * KKn                     # columns per gather group (256)

    nc = bacc.Bacc()

    # ---- DRAM parameters ----
    key_r = nc.declare_dram_parameter("key_r", [Bn, CINn, NLOCn * KKn], bf16, isOutput=False)
    val_r = nc.declare_dram_parameter("val_r", [Bn, CINn, NLOCn * KKn], bf16, isOutput=False)
    paug_lhs = nc.declare_dram_parameter("paug_lhs", [Bn, 11, NLOCn], bf16, isOutput=False)
    paug_rhs = nc.declare_dram_parameter("paug_rhs", [Bn, 11, Nn], bf16, isOutput=False)
    pos128_d = nc.declare_dram_parameter("pos128", [Bn, 128, Nn], fp32, isOutput=False)
    WqTbG_d = nc.declare_dram_parameter("WqTbG", [128, GP, DM, 128], bf16, isOutput=False)
    Wp1qG_d = nc.declare_dram_parameter("Wp1qG", [128, GP, PHIDn], bf16, isOutput=False)
    WkTnN_d = nc.declare_dram_parameter("WkTnN", [CINn, DIMn], bf16, isOutput=False)
    Wp2T_d = nc.declare_dram_parameter("Wp2T", [PHIDn, DIMn], bf16, isOutput=False)
    WvT_d = nc.declare_dram_parameter("WvT", [CINn, DIMn], bf16, isOutput=False)
    Wa1T_d = nc.declare_dram_parameter("Wa1T", [128, KA1, AHIDn], bf16, isOutput=False)
    Wa2T_d = nc.declare_dram_parameter("Wa2T", [128, AM, DIMn], bf16, isOutput=False)
    WeT_d = nc.declare_dram_parameter("WeT", [128, DM, DIMn], bf16, isOutput=False)
    bp2_d = nc.declare_dram_parameter("bp2f", [128, DM], fp32, isOutput=False)
    ba1_d = nc.declare_dram_parameter("ba1f", [128, AM], fp32, isOutput=False)
    be_d = nc.declare_dram_parameter("bef", [128, DM], fp32, isOutput=False)
    eye_d = nc.declare_dram_parameter("eye128", [128, 128], bf16, isOutput=False)
    out_d = nc.declare_dram_parameter("out", [Bn, DIMn, NLOCn], fp32, isOutput=True)

    with tile.TileContext(nc) as tc:
        with (
            tc.tile_pool(name="wpool", bufs=1) as wpool,
            tc.tile_pool(name="papool", bufs=2) as papool,
            tc.tile_pool(name="dpool", bufs=2) as dpool,
            tc.tile_pool(name="kpool", bufs=2) as kpool,
            tc.tile_pool(name="iopool", bufs=3) as iopool,
            tc.tile_pool(name="gpool", bufs=2) as gpool,
            tc.tile_pool(name="cpool", bufs=2) as cpool,
            tc.tile_pool(name="bpool", bufs=2) as bpool,
            tc.tile_pool(name="ypool", bufs=2) as ypool,
            tc.tile_pool(name="psbig", bufs=2, space="PSUM") as psbig,
            tc.tile_pool(name="pssm", bufs=2, space="PSUM") as pssm,
            tc.tile_pool(name="psvp", bufs=2, space="PSUM") as psvp,
        ):
            # ---- per-batch tables first (they gate the first distance
            #      matmuls), then the weights ----
            prhs_sbs, plhs_sbs, pos128s = [], [], []
            for b in range(Bn):
                prhs_sb = papool.tile([11, Nn], bf16, tag="prhs_sb")
                nc.sync.dma_start(out=prhs_sb[:], in_=paug_rhs[b])
                plhs_sb = papool.tile([11, NLOCn], bf16, tag="plhs_sb")
                nc.sync.dma_start(out=plhs_sb[:], in_=paug_lhs[b])
                pos128 = papool.tile([128, Nn], fp32, tag="pos128")
                nc.sync.dma_start(out=pos128[:], in_=pos128_d[b])
                prhs_sbs.append(prhs_sb); plhs_sbs.append(plhs_sb)
                pos128s.append(pos128)

            WqTbG = wpool.tile([128, GP, DM, 128], bf16)
            Wp1qG = wpool.tile([128, GP, PHIDn], bf16)
            WkTnN = wpool.tile([CINn, DIMn], bf16)
            Wp2T = wpool.tile([PHIDn, DIMn], bf16)
            WvT = wpool.tile([CINn, DIMn], bf16)
            Wa1T = wpool.tile([128, KA1, AHIDn], bf16)
            Wa2T = wpool.tile([128, AM, DIMn], bf16)
            WeT = wpool.tile([128, DM, DIMn], bf16)
            bp2f = wpool.tile([128, DM], fp32)
            ba1f = wpool.tile([128, AM], fp32)
            bef = wpool.tile([128, DM], fp32)
            eye128 = wpool.tile([128, 128], bf16)
            for sb, dr in [(WqTbG, WqTbG_d), (Wp1qG, Wp1qG_d),
                           (WkTnN, WkTnN_d), (Wp2T, Wp2T_d), (WvT, WvT_d),
                           (Wa1T, Wa1T_d), (Wa2T, Wa2T_d), (WeT, WeT_d),
                           (bp2f, bp2_d), (ba1f, ba1_d), (bef, be_d),
                           (eye128, eye_d)]:
                nc.sync.dma_start(out=sb[:], in_=dr[:])
            zeros16 = wpool.tile([128, 128], u16)
            nc.vector.memset(zeros16[:], 0)

            def stt_bit(out, in0, imm, in1, op0, op1):
                """scalar_tensor_tensor with a uint16 bitwise immediate."""
                return nc.vector.add_instruction(
                    mybir.InstTensorScalarPtr(
                        name=nc.vector.bass.get_next_instruction_name(),
                        is_scalar_tensor_tensor=True,
                        op0=op0, op1=op1,
                        ins=[nc.vector.lower_ap(in0),
                             mybir.ImmediateValue(dtype=u16, value=imm),
                             nc.vector.lower_ap(in1)],
                        outs=[nc.vector.lower_ap(out)]))

            def knn_start(b, t):
                """Allocate tiles and return (posgb, pieces); pieces are
                emitted interleaved between chunks so the in-order DVE isn't
                head-of-line blocked by the KNN scans."""
                dsb = dpool.tile([QT, Nn], fp32, tag="dsb")
                v8a = kpool.tile([QT, 8], fp32, tag="v8a")
                v8b = kpool.tile([QT, 8], fp32, tag="v8b")
                idxg = kpool.tile([QT, 16], u16, tag="idxg")
                idxgB = kpool.tile([QT, 16], u16, tag="idxgB")
                idxwbf = kpool.tile([16, QT], bf16, tag="idxwbf")
                idxw16 = kpool.tile([16, QT], u16, tag="idxw16")
                idxB = kpool.tile([QT, 16], u16, tag="idxB")
                posg = gpool.tile([GP * 16, GW], fp32, tag="posg")
                posgb = gpool.tile([GP * 16, GW], bf16, tag="posgb")

                def p_d(lo, hi):
                    for nch in range(lo, hi):
                        dps = pssm.tile([QT, 512], fp32, tag="sm")
                        nc.tensor.matmul(
                            dps[:], plhs_sbs[b][:, ts(t, QT)],
                            prhs_sbs[b][:, ts(nch, 512)])
                        nc.scalar.activation(dsb[:, ts(nch, 512)], dps[:],
                                             AF.Copy)

                def p1():
                    nc.vector.max(out=v8a[:], in_=dsb[:])
                    nc.vector.max_index(out=idxg[:, 0:8], in_max=v8a[:],
                                        in_values=dsb[:])

                def p2():
                    nc.vector.match_replace(out=dsb[:], in_to_replace=v8a[:],
                                            in_values=dsb[:],
                                            imm_value=NEG_BIG)
                    nc.vector.max(out=v8b[:], in_=dsb[:])

                def p3():
                    nc.vector.max_index(out=idxg[:, 8:16], in_max=v8b[:],
                                        in_values=dsb[:])
                    # block-transpose the [16q x 16slot] index blocks on the
                    # PE so each Q7 core sees its group's indices in its own
                    # 16 partitions.  u16 indices are biased by 0x4000 so
                    # their bf16 bit patterns are normal numbers in [2, 4)
                    # that survive the matmul-with-identity bit-exactly.
                    stt_bit(idxgB[:], idxg[:], 0x4000, zeros16[0:QT, 0:16],
                            OP.bitwise_or, OP.bitwise_or)
                    tps = pssm.tile([16, 256], bf16, tag="sm")
                    nc.tensor.transpose(tps[:, 0:QT], idxgB[:].bitcast(bf16),
                                        eye128[0:QT, 0:QT])
                    nc.scalar.activation(idxwbf[:], tps[:, 0:QT], AF.Copy)
                    stt_bit(idxw16[:], idxwbf[:].bitcast(u16), 0x0FFF,
                            zeros16[0:16, 0:QT],
                            OP.bitwise_and, OP.bitwise_or)
                    # replicate [slot, q] into per-Q7-core partition groups:
                    # group g's 16 slot rows land at partitions 16g..16g+16
                    # with its 16 queries along the free dim (tiny DMAs do
                    # the partition-crossing move)
                    for g2 in range(GP):
                        nc.sync.dma_start(
                            out=idxB[g2 * 16:(g2 + 1) * 16, :],
                            in_=idxw16[0:16, g2 * 16:(g2 + 1) * 16])
                    # ONE 8-core ap_gather for the whole tile: core g reads
                    # its 16 index partitions and fetches its group's 256
                    # (query,slot) position columns from its replica of the
                    # position table.
                    nc.gpsimd.ap_gather(
                        posg[:], pos128s[b][0:GP * 16, :],
                        idxB[:].bitcast(i16),
                        channels=GP * 16, num_elems=Nn, d=1, num_idxs=GW)
                    nc.scalar.activation(posgb[:], posg[:], AF.Copy)

                nd2 = NCH_D // 2
                return posgb, [lambda: p_d(0, nd2), lambda: p_d(nd2, NCH_D),
                               p1, p2, p3]

            # global chunk order; IO (key/value chunk DMA) issued one ahead
            tiles = [(b, t) for b in range(Bn) for t in range(NQT)]
            chunk_list = [(b, t, c) for (b, t) in tiles
                          for c in range(NCH_TILE)]

            def start_io(ci):
                if ci >= len(chunk_list):
                    return None
                b, t, c = chunk_list[ci]
                col0 = (t * NCH_TILE + c) * CHUNK
                kbf = iopool.tile([CINn, CHUNK], bf16, tag="kbf")
                vbf = iopool.tile([CINn, CHUNK], bf16, tag="vbf")
                nc.sync.dma_start(out=kbf[:],
                                  in_=key_r[b, :, col0:col0 + CHUNK])
                nc.sync.dma_start(out=vbf[:],
                                  in_=val_r[b, :, col0:col0 + CHUNK])
                return kbf, vbf

            def emit_chunk(b, t, c, io, posgb, aggsb):
                kbf, vbf = io
                pmv = posgb[:, 0:GW]           # moving operand, all groups

                # ---- rr = Wq@pos + (bq-bk+1) - Wk@key, all in PSUM ----
                # (per-group stationary weights select the right rows)
                rps = psbig.tile([128, DM, CHUNK], fp32, tag="big")
                for m in range(DM):
                    for h in range(2):
                        nc.tensor.matmul(
                            rps[:, m, h * GW:(h + 1) * GW],
                            WqTbG[0:GP * 16, 2 * c + h, m, :], pmv,
                            start=(h == 0), stop=False)
                for m in range(DM):
                    nc.tensor.matmul(rps[:, m, :], WkTnN[:, ts(m, 128)],
                                     kbf[:], start=False, stop=True)

                # ---- pe path: p1 composed with Wq acts on positions ----
                p1ps = pssm.tile([PHIDn, 512], fp32, tag="sm")
                for h in range(2):
                    nc.tensor.matmul(
                        p1ps[:, h * GW:(h + 1) * GW],
                        Wp1qG[0:GP * 16, 2 * c + h, :], pmv,
                        start=(h == 0), stop=(h == 1))
                pe1c = cpool.tile([PHIDn, CHUNK], bf16, tag="pe1c")
                nc.scalar.activation(pe1c[:], p1ps[:, 0:CHUNK], AF.Relu)

                # ---- pe for a1in: own p2 group so it can be evicted ----
                peg = cpool.tile([128, DM, CHUNK], bf16, tag="peg")
                for m in range(DM):
                    p2ps = pssm.tile([128, 512], fp32, tag="sm")
                    nc.tensor.matmul(p2ps[:, 0:CHUNK], Wp2T[:, ts(m, 128)],
                                     pe1c[:])
                    nc.scalar.activation(peg[:, m, :], p2ps[:, 0:CHUNK],
                                         AF.Identity, bias=bp2f[:, m:m + 1])
                # ---- vpe = Wp2@pe1 + Wv@val accumulated in PSUM ----
                vpems = []
                for m in range(DM):
                    vpem = psvp.tile([128, CHUNK], fp32, tag="vp")
                    nc.tensor.matmul(vpem[:], Wp2T[:, ts(m, 128)], pe1c[:],
                                     start=True, stop=False)
                    nc.tensor.matmul(vpem[:], WvT[:, ts(m, 128)], vbf[:],
                                     start=False, stop=True)
                    vpems.append(vpem)

                # ---- a1in = rr * pe (first DVE op: frees the rps slot) ----
                a1in = cpool.tile([128, DM, CHUNK], bf16, tag="a1in")
                nc.vector.tensor_mul(a1in[:], rps[:], peg[:])

                # ---- a-branch MLP ----
                a1r = cpool.tile([128, AM, CHUNK], bf16, tag="a1r")
                for mt in range(AM):
                    a1ps = pssm.tile([128, 512], fp32, tag="sm")
                    for kt in range(KA1):
                        nc.tensor.matmul(
                            a1ps[:, 0:CHUNK], Wa1T[:, kt, ts(mt, 128)],
                            a1in[:, kt, :],
                            start=(kt == 0), stop=(kt == KA1 - 1))
                    nc.scalar.activation(a1r[:, mt, :], a1ps[:, 0:CHUNK],
                                         AF.Relu, bias=ba1f[:, mt:mt + 1])

                a2ps = psbig.tile([128, DM, CHUNK], fp32, tag="big")
                for m in range(DM):
                    for kt in range(AM):
                        nc.tensor.matmul(
                            a2ps[:, m, :], Wa2T[:, kt, ts(m, 128)],
                            a1r[:, kt, :],
                            start=(kt == 0), stop=(kt == AM - 1))

                # ---- softmax over the 16 neighbours: sum(e*w)/sum(e) ----
                ee = cpool.tile([128, DM, CHUNK], bf16, tag="ee")
                nc.scalar.activation(ee[:], a2ps[:], AF.Exp)
                # eev first: frees the vpem slots for the next chunk's p2
                eev = cpool.tile([128, DM, CHUNK], fp32, tag="eev")
                for m in range(DM):
                    nc.vector.tensor_tensor(eev[:, m, :], ee[:, m, :],
                                            vpems[m][:], op=OP.mult)
                esum = cpool.tile([128, DM, CQ], fp32, tag="esum")
                nc.vector.tensor_reduce(
                    esum[:],
                    ee[:].rearrange("p m (g k) -> p m g k", k=KKn),
                    axis=AX.X, op=OP.add)
                erec = cpool.tile([128, DM, CQ], fp32, tag="erec")
                nc.vector.reciprocal_approx_fast(erec[:], esum[:])
                aggc = cpool.tile([128, DM, CQ], fp32, tag="aggc")
                nc.vector.tensor_reduce(
                    aggc[:],
                    eev[:].rearrange("p m (g k) -> p m g k", k=KKn),
                    axis=AX.X, op=OP.add)
                nc.vector.tensor_mul(
                    aggsb[:, :, t * QT + c * CQ:t * QT + (c + 1) * CQ],
                    aggc[:], erec[:])

            # ---- main loop: chunks with KNN pieces interleaved ----
            # pieces for tile t+1 are shifted one chunk early (distance
            # matmuls at tile start, scans after chunks 0..NCH_TILE-2) so
            # the tile boundary never waits on the scans or the gather.
            posgb, pieces = knn_start(*tiles[0])
            for p in pieces:
                p()
            io = start_io(0)
            aggsb = None
            ci = 0
            for ti, (b, t) in enumerate(tiles):
                if t == 0:
                    aggsb = bpool.tile([128, DM, NLOCn], bf16, tag="aggsb")
                if ti + 1 < len(tiles):
                    next_posgb, pieces = knn_start(*tiles[ti + 1])
                    pieces[0]()               # half the distance matmuls at
                    pieces = pieces[1:]       # tile start, rest spread over
                else:                         # the chunk boundaries
                    next_posgb, pieces = None, []
                for c in range(NCH_TILE):
                    next_io = start_io(ci + 1)
                    emit_chunk(b, t, c, io, posgb, aggsb)
                    if c < len(pieces):
                        pieces[c]()
                    if c == NCH_TILE - 1:
                        for p in pieces[NCH_TILE:]:
                            p()
                    io = next_io
                    ci += 1
                posgb = next_posgb

                # ---- final 1x1 conv once this batch's tiles are done ----
                if t == NQT - 1:
                    for nloc0 in range(0, NLOCn, 512):
                        w = min(512, NLOCn - nloc0)
                        for m in range(DM):
                            yps = pssm.tile([128, 512], fp32, tag="sm")
                            for kt in range(DM):
                                nc.tensor.matmul(
                                    yps[:, :w], WeT[:, kt, ts(m, 128)],
                                    aggsb[:, kt, nloc0:nloc0 + w],
                                    start=(kt == 0), stop=(kt == DM - 1))
                            yev = ypool.tile([128, 512], fp32, tag="yev")
                            nc.scalar.activation(yev[:, :w], yps[:, :w],
                                                 AF.Identity,
                                                 bias=bef[:, m:m + 1])
                            nc.sync.dma_start(
                                out=out_d[b, ts(m, 128), nloc0:nloc0 + w],
                                in_=yev[:, :w])

    nc.finalize()   # Bacc.finalize: wait legalization, library loads, ISA codegen
    return nc


def host_prepare(inputs, dims, ncores=NCORES):
    """Fold BN/biases into weights, pre-transpose for the PE, shard by n."""
    d = dims
    f32 = np.float32
    key = np.asarray(inputs["key"], f32)
    values = np.asarray(inputs["values"], f32)
    pos = np.asarray(inputs["pos"], f32)
    g = lambda n: np.asarray(inputs[n], f32)

    Wk, bk = g("Wk"), g("bk")
    Wq, bq = g("Wq"), g("bq")
    Wv, bv = g("Wv"), g("bv")
    Wp1, bp1 = g("Wp1"), g("bp1")
    Wp2, bp2 = g("Wp2"), g("bp2")
    Wa1, ba1 = g("Wa1"), g("ba1")
    Wa2 = g("Wa2")
    We, be = g("We"), g("be")

    p_sc = g("p_gamma") / np.sqrt(g("p_var") + f32(BN_EPS))
    Wp1f = (Wp1 * p_sc[:, None]).astype(f32)
    bp1f = (bp1 * p_sc + g("p_beta") - g("p_mean") * p_sc).astype(f32)
    a_sc = g("a_gamma") / np.sqrt(g("a_var") + f32(BN_EPS))
    Wa1f = (Wa1 * a_sc[:, None]).astype(f32)
    ba1f = (ba1 * a_sc + g("a_beta") - g("a_mean") * a_sc).astype(f32)

    # compose the pe-path first layer with the q conv: acts on positions
    Wp1q = (Wp1f @ Wq).astype(f32)                    # (PHID, 3)
    bp1q = (Wp1f @ bq + bp1f).astype(f32)             # (PHID,)

    DM = d["DIM"] // 128
    AM = d["AHID"] // 128
    KA1 = d["DIM"] // 128
    QT = min(128, d["NLOC"])
    GP = QT // 16

    def colsplit(v, nt):  # (nt*128,) -> (128, nt)
        return np.ascontiguousarray(v.reshape(nt, 128).T).astype(f32)

    # per-group stationary weights: group g's 4 position rows live at
    # partitions 16g..16g+4 of the moving operand
    WqTb = np.concatenate([Wq.T, (bq - bk + 1.0)[None, :]], 0)  # (4, DIM)
    Wp1qT = np.concatenate([Wp1q.T, bp1q[None, :]], 0)          # (4, PHID)
    WqTbG = np.zeros((128, GP, DM, 128), f32)
    Wp1qG = np.zeros((128, GP, d["PHID"]), f32)
    for gidx in range(GP):
        for m in range(DM):
            WqTbG[16 * gidx:16 * gidx + 4, gidx, m, :] = \
                WqTb[:, m * 128:(m + 1) * 128]
        Wp1qG[16 * gidx:16 * gidx + 4, gidx, :] = Wp1qT

    common = {
        "WqTbG": WqTbG.astype(BF16),
        "Wp1qG": Wp1qG.astype(BF16),
        "WkTnN": np.ascontiguousarray(-Wk.T).astype(BF16),
        "Wp2T": np.ascontiguousarray(Wp2.T).astype(BF16),
        "WvT": np.ascontiguousarray(Wv.T).astype(BF16),
        "Wa1T": np.ascontiguousarray(
            Wa1f.T.reshape(KA1, 128, d["AHID"]).transpose(1, 0, 2)).astype(BF16),
        "Wa2T": np.ascontiguousarray(
            Wa2.T.reshape(AM, 128, d["DIM"]).transpose(1, 0, 2)).astype(BF16),
        "WeT": np.ascontiguousarray(
            We.T.reshape(DM, 128, d["DIM"]).transpose(1, 0, 2)).astype(BF16),
        "bp2f": colsplit(bp2, DM),
        "ba1f": colsplit(ba1f, AM),
        "bef": colsplit((We @ (bv + bp2) + be).astype(f32), DM),
        "eye128": np.eye(128, dtype=np.float32).astype(BF16),
    }

    # hi/lo bf16 split of pos and |p|^2 for the exact-enough distance matmul:
    # dneg = 2(hi_i+lo_i).(hi_j+lo_j) - sq_j, dropping only the lo.lo term.
    sq = (pos * pos).sum(axis=1).astype(f32)              # (B, N)
    pos_hi = pos.astype(BF16)
    pos_lo = (pos - pos_hi.astype(f32)).astype(BF16)
    sq_hi = sq.astype(BF16)
    sq_lo = (sq - sq_hi.astype(f32)).astype(BF16)
    paug_rhs = np.concatenate(
        [2.0 * pos_hi.astype(f32), 2.0 * pos_lo.astype(f32),
         2.0 * pos_hi.astype(f32), -sq_hi.astype(f32)[:, None, :],
         -sq_lo.astype(f32)[:, None, :]], 1).astype(BF16)
    # position table replicated into all 8 Q7-core partition groups
    pos16 = np.zeros((d["B"], 16, d["N"]), f32)
    pos16[:, 0:3] = pos
    pos16[:, 3] = 1.0
    pos128 = np.tile(pos16, (1, 8, 1))

    in_maps = []
    for cid in range(ncores):
        n0 = cid * d["NLOC"]
        n1 = n0 + d["NLOC"]
        m = dict(common)
        m["key_r"] = np.ascontiguousarray(key[:, :, n0:n1, :]).reshape(
            d["B"], d["CIN"], d["NLOC"] * d["KK"]).astype(BF16)
        m["val_r"] = np.ascontiguousarray(values[:, :, n0:n1, :]).reshape(
            d["B"], d["CIN"], d["NLOC"] * d["KK"]).astype(BF16)
        m["paug_lhs"] = np.ascontiguousarray(np.concatenate(
            [pos_hi.astype(f32)[:, :, n0:n1], pos_hi.astype(f32)[:, :, n0:n1],
             pos_lo.astype(f32)[:, :, n0:n1],
             np.ones((d["B"], 2, d["NLOC"]), f32)], 1)).astype(BF16)
        m["paug_rhs"] = paug_rhs
        m["pos128"] = pos128
        in_maps.append(m)
    return in_maps


_NC_CACHE = {}


def _get_nc(dims_key):
    if dims_key not in _NC_CACHE:
        _NC_CACHE[dims_key] = build_nc(_dims_full())
    return _NC_CACHE[dims_key]


def kernel(**inputs):
    from concourse.bass_utils import run_bass_kernel_spmd
    dims = _dims_full()
    nc = _get_nc("full")
    in_maps = host_prepare(inputs, dims)
    res = run_bass_kernel_spmd(nc, in_maps, core_ids=list(range(NCORES)))
    outs = [r["out"].astype(np.float32) for r in res.results]
    return np.concatenate(outs, axis=2)
